# revision 29
# baseline (speedup 1.0000x reference)
"""NeRF-style positional encoding kernel for Trainium2 (8 NeuronCores).

out[n, 2j]   = cos(x[n] * freqs[j])
out[n, 2j+1] = sin(x[n] * freqs[j])     freqs[j] = fl(pi * exp2(j)) as the
                                        neuron device computes them.

Bit-exact replication of the neuronx-cc lowering of jnp.sin/jnp.cos:
    sin(v): t = RN(RN(v + PI) * INV2PI); k = floor(t)  [rne-convert + is_gt fixup]
            r = RN(v + RN(k * -TWOPI)); out = ActSin(r)
    cos(v): same chain applied to y = RN(v + HALFPI)

Sharding: pure data-parallel across 8 cores along n.
"""
import numpy as np

import concourse.bacc as bacc
import concourse.tile as tile
import concourse.mybir as mybir
from concourse.bass_utils import run_bass_kernel_spmd

N_TOTAL = 4194304
N_CORES = 8
N_PER_CORE = N_TOTAL // N_CORES     # 524288
D = 32

P = 128                             # partitions
F = 128                             # x elements per partition per tile
TILE_ELEMS = P * F                  # 32768
N_TILES = N_PER_CORE // TILE_ELEMS  # 16
G = 8                               # j's per group
N_GROUPS = D // G                   # 4
GF = G * F                          # 2048

# Device-computed freqs bits (pi * exp2(arange(32)) evaluated by neuronx-cc
# on trn2 -- the exp2 ACT table is not exact, so these differ from
# fl(pi)*2^j for most j).
FREQS_BITS = [
    1078530011, 1086918608, 1095307227, 1103695832, 1112084378, 1120473016,
    1128861658, 1137250267, 1145638851, 1154027401, 1162416086, 1170804699,
    1179193302, 1187581820, 1195970503, 1204359125, 1212747739, 1221136313,
    1229524901, 1237913555, 1246302171, 1254690773, 1263079269, 1271467979,
    1279856603, 1288245214, 1296633765, 1305022388, 1313411035, 1321799649,
    1330188214, 1338576773,
]
FREQS = np.array(FREQS_BITS, dtype=np.uint32).view(np.float32)

PI = float(np.float32(3.1415927410125732))       # 0x40490FDB
INV2PI = float(np.float32(0.15915493667125702))  # 0x3E22F983
NTWOPI = float(np.float32(-6.2831854820251465))  # 0xC0C90FDB
HALFPI = float(np.float32(1.5707963705062866))   # 0x3FC90FDB

_NC_CACHE = {}


def _register_floorfix():
    """Register the FLOORFIX custom DVE op: out = f32(in0) - (f32(in0) > in1).

    Fuses the reference's rne->floor fixup (is_gt + select) into one DVE
    instruction. in0 = rne-converted int32 k, in1 = the f32 t it came from.
    """
    import concourse.dve_ops as dops
    from concourse.dve_spec import Spec, Src0, Src1, lower
    from concourse.dve_uop import DveOpSpec
    from concourse.dve_table_gen import dve_ver_for

    name = "FLOORFIX_ANT"
    for o in dops.OPS:
        if o.name == name:
            return o
    spec = Spec(
        body=Src0 - (Src0 > Src1),
        reference=lambda in0, in1, s0, s1, imm2: (
            in0.astype(np.float32)
            - (in0.astype(np.float32) > in1).astype(np.float32)
        ).astype(np.float32),
    )
    shas = {}
    for ver in ("v3", "v4"):
        tmp = DveOpSpec(name=name, opcode=0, uops=lower(spec, ver=ver), rd1_en=True)
        shas[ver] = tmp.sha(ver)
    op = dops.DveOp(name, spec, subdim=False, uops_sha=shas)
    dops.OPS.append(op)
    dops.CUSTOM_DVE_SPECS[name] = spec
    dops._SUB_OPCODE_FOR_NAME[name] = dops._CUSTOM_DVE_ROW_BASE + len(dops.OPS) - 1
    return op


FLOORFIX = _register_floorfix()


def _emit_tile(nc, pools, xt, ot, f=F):
    """Emit the full 64-column computation for one [P, f] x-tile into the
    [P, f*64] out tile. The sin (src=ang) and cos (src=y) chains are emitted
    stage-interleaved so DVE/GP/ACT overlap."""
    f32 = mybir.dt.float32
    i32 = mybir.dt.int32
    A = mybir.AluOpType
    Sin = mybir.ActivationFunctionType.Sin
    Ident = mybir.ActivationFunctionType.Identity
    src_pool, work_pool, rm_pool, hp = (
        pools["src"], pools["work"], pools["rm"], pools["hp"])
    gf = G * f

    angs, ys = {}, {}
    for g in range(N_GROUPS):
        ang = src_pool.tile([P, gf], f32, tag="src", name="ang%d" % g)
        for i in range(G):
            # NB: single-scalar tensor_scalar on Pool crashes the device;
            # always use the two-op form there.
            if i < 2:
                nc.vector.tensor_scalar_mul(
                    ang[:, i * f:(i + 1) * f], xt[:], float(FREQS[g * G + i]))
            else:
                nc.gpsimd.tensor_scalar(
                    ang[:, i * f:(i + 1) * f], xt[:], 0.0,
                    float(FREQS[g * G + i]), A.add, A.mult)
        y = src_pool.tile([P, gf], f32, tag="src", name="y%d" % g)
        nc.scalar.activation(y[:], ang[:], Ident, bias=hp[:], scale=1.0)
        angs[g], ys[g] = ang, y

    for g in range(N_GROUPS):
        srcs = (angs[g], ys[g])
        ov = ot[:].rearrange("p (c e two) -> p two e c", e=D, two=2)
        t_, ki_, kfix_, r_ = {}, {}, {}, {}
        for s in (0, 1):
            t_[s] = work_pool.tile([P, gf], f32, tag="work", name="t%d" % s)
            nc.gpsimd.tensor_scalar(t_[s][:], srcs[s][:], PI, INV2PI,
                                    A.add, A.mult)
        for s in (0, 1):
            ki_[s] = work_pool.tile([P, gf], i32, tag="work", name="ki%d" % s)
            nc.vector.tensor_copy(ki_[s][:], t_[s][:])
        for s in (0, 1):
            kfix_[s] = work_pool.tile([P, gf], f32, tag="work", name="kfix%d" % s)
            nc.vector._custom_dve(FLOORFIX, out=kfix_[s][:], in0=ki_[s][:],
                                  in1=t_[s][:])
        for s in (0, 1):
            r_[s] = work_pool.tile([P, gf], f32, tag="work", name="r%d" % s)
            nc.vector.scalar_tensor_tensor(r_[s][:], kfix_[s][:], NTWOPI,
                                           srcs[s][:], A.mult, A.add)
        for s in (0, 1):
            # sin of ang (s=0) -> odd cols; sin of y (s=1) = cos -> even cols
            nc.scalar.activation(ov[:, 1 - s, g * G:(g + 1) * G, :],
                                 r_[s][:].rearrange("p (e c) -> p e c", e=G),
                                 Sin)


def _emit_kernel(nc, tc, x_v, out_v, n_tiles, repeats=1, loop=None, tiny_ap=None,
                 f=F, bufs_work=16, bufs_src=8):
    f32 = mybir.dt.float32
    with (
        tc.tile_pool(name="io", bufs=2) as io_pool,
        tc.tile_pool(name="src", bufs=bufs_src) as src_pool,
        tc.tile_pool(name="work", bufs=bufs_work) as work_pool,
        tc.tile_pool(name="rm", bufs=3) as rm_pool,
        tc.tile_pool(name="cst", bufs=1) as cst_pool,
    ):
        hp = cst_pool.tile([P, 1], f32, tag="hp")
        nc.gpsimd.memset(hp[:], HALFPI)
        pools = {"src": src_pool, "work": work_pool, "rm": rm_pool, "hp": hp}

        def body():
            xall = io_pool.tile([P, n_tiles * f], f32, tag="xall", bufs=1)
            nc.sync.dma_start(xall[:], x_v)
            for ti in range(n_tiles * repeats):
                tix = ti % n_tiles
                ot = io_pool.tile([P, f * 2 * D], f32, tag="out")
                _emit_tile(nc, pools, xall[:, tix * f:(tix + 1) * f], ot, f=f)
                dma_eng = nc.sync if tix % 2 == 0 else nc.scalar
                dma_eng.dma_start(out_v[tix], ot[:])

        if loop is None:
            body()
        else:
            with tc.For_i(0, loop, 1):
                body()
            xd = io_pool.tile([P, 1], f32, tag="xd")
            nc.vector.memset(xd[:], 0.0)
            nc.sync.dma_start(tiny_ap, xd[:])


def build_nc(repeats: int = 1, n_tiles: int | None = None, n_devices: int = N_CORES,
             f: int = F, bufs_work: int = 16, bufs_src: int = 8):
    f32 = mybir.dt.float32
    if n_tiles is None:
        n_tiles = N_PER_CORE // (P * f)
    n_elems = n_tiles * P * f
    nc = bacc.Bacc("TRN2", target_bir_lowering=False, debug=False,
                   num_devices=n_devices)
    x = nc.dram_tensor("x", [n_elems], f32, kind="ExternalInput")
    out = nc.dram_tensor("out", [n_elems, 2 * D], f32, kind="ExternalOutput")
    x_v = x.ap().rearrange("(p w) -> p w", p=P)
    out_v = out.ap().rearrange("(p t c) q -> t p (c q)", p=P, t=n_tiles)
    with tile.TileContext(nc) as tc:
        _emit_kernel(nc, tc, x_v, out_v, n_tiles, repeats=repeats,
                     f=f, bufs_work=bufs_work, bufs_src=bufs_src)
    nc.compile()
    return nc


def build_timing_nc(loop_iters: int, n_tiles: int | None = None,
                    f: int = F, bufs_work: int = 16, bufs_src: int = 8):
    """Timing variant: out goes to internal DRAM; whole pass loops on-device;
    only a tiny dummy tensor is fetched back."""
    f32 = mybir.dt.float32
    if n_tiles is None:
        n_tiles = N_PER_CORE // (P * f)
    n_elems = n_tiles * P * f
    nc = bacc.Bacc("TRN2", target_bir_lowering=False, debug=False,
                   num_devices=N_CORES)
    x = nc.dram_tensor("x", [n_elems], f32, kind="ExternalInput")
    out = nc.dram_tensor("scratch_out", [n_elems, 2 * D], f32)
    tiny = nc.dram_tensor("tiny_out", [P, 1], f32, kind="ExternalOutput")
    x_v = x.ap().rearrange("(p w) -> p w", p=P)
    out_v = out.ap().rearrange("(p t c) q -> t p (c q)", p=P, t=n_tiles)
    with tile.TileContext(nc) as tc:
        _emit_kernel(nc, tc, x_v, out_v, n_tiles, loop=loop_iters, tiny_ap=tiny.ap(),
                     f=f, bufs_work=bufs_work, bufs_src=bufs_src)
    nc.compile()
    return nc


def _get_nc(repeats: int = 1):
    if repeats not in _NC_CACHE:
        _NC_CACHE[repeats] = build_nc(repeats)
    return _NC_CACHE[repeats]


def kernel(x, d):
    assert int(d) == D
    x = np.ascontiguousarray(np.asarray(x, dtype=np.float32).reshape(N_TOTAL))
    xs = x.reshape(N_CORES, N_PER_CORE)
    nc = _get_nc()
    res = run_bass_kernel_spmd(
        nc, [{"x": xs[i]} for i in range(N_CORES)], core_ids=list(range(N_CORES)))
    out = np.empty((N_TOTAL, 2 * D), dtype=np.float32)
    for i in range(N_CORES):
        out[i * N_PER_CORE:(i + 1) * N_PER_CORE] = res.results[i]["out"]
    return out


# revision 30
# speedup vs baseline: 1.2104x; 1.2104x over previous
"""NeRF-style positional encoding kernel for Trainium2 (8 NeuronCores).

out[n, 2j]   = cos(x[n] * freqs[j])
out[n, 2j+1] = sin(x[n] * freqs[j])     freqs[j] = fl(pi * exp2(j)) as the
                                        neuron device computes them.

Bit-exact replication of the neuronx-cc lowering of jnp.sin/jnp.cos:
    sin(v): t = RN(RN(v + PI) * INV2PI); k = floor(t)  [rne-convert + is_gt fixup]
            r = RN(v + RN(k * -TWOPI)); out = ActSin(r)
    cos(v): same chain applied to y = RN(v + HALFPI)

Sharding: pure data-parallel across 8 cores along n.
"""
import numpy as np

import concourse.bacc as bacc
import concourse.tile as tile
import concourse.mybir as mybir
from concourse.bass_utils import run_bass_kernel_spmd

N_TOTAL = 4194304
N_CORES = 8
N_PER_CORE = N_TOTAL // N_CORES     # 524288
D = 32

P = 128                             # partitions
F = 128                             # x elements per partition per tile
TILE_ELEMS = P * F                  # 32768
N_TILES = N_PER_CORE // TILE_ELEMS  # 16
G = 8                               # j's per group
N_GROUPS = D // G                   # 4
GF = G * F                          # 2048

# Device-computed freqs bits (pi * exp2(arange(32)) evaluated by neuronx-cc
# on trn2 -- the exp2 ACT table is not exact, so these differ from
# fl(pi)*2^j for most j).
FREQS_BITS = [
    1078530011, 1086918608, 1095307227, 1103695832, 1112084378, 1120473016,
    1128861658, 1137250267, 1145638851, 1154027401, 1162416086, 1170804699,
    1179193302, 1187581820, 1195970503, 1204359125, 1212747739, 1221136313,
    1229524901, 1237913555, 1246302171, 1254690773, 1263079269, 1271467979,
    1279856603, 1288245214, 1296633765, 1305022388, 1313411035, 1321799649,
    1330188214, 1338576773,
]
FREQS = np.array(FREQS_BITS, dtype=np.uint32).view(np.float32)

PI = float(np.float32(3.1415927410125732))       # 0x40490FDB
INV2PI = float(np.float32(0.15915493667125702))  # 0x3E22F983
NTWOPI = float(np.float32(-6.2831854820251465))  # 0xC0C90FDB
HALFPI = float(np.float32(1.5707963705062866))   # 0x3FC90FDB

_NC_CACHE = {}


def _register_floorfix():
    """Register the FLOORFIX custom DVE op: out = f32(in0) - (f32(in0) > in1).

    Fuses the reference's rne->floor fixup (is_gt + select) into one DVE
    instruction. in0 = rne-converted int32 k, in1 = the f32 t it came from.
    """
    import concourse.dve_ops as dops
    from concourse.dve_spec import Spec, Src0, Src1, lower
    from concourse.dve_uop import DveOpSpec
    from concourse.dve_table_gen import dve_ver_for

    name = "FLOORFIX_ANT"
    for o in dops.OPS:
        if o.name == name:
            return o
    spec = Spec(
        body=Src0 - (Src0 > Src1),
        reference=lambda in0, in1, s0, s1, imm2: (
            in0.astype(np.float32)
            - (in0.astype(np.float32) > in1).astype(np.float32)
        ).astype(np.float32),
    )
    shas = {}
    for ver in ("v3", "v4"):
        tmp = DveOpSpec(name=name, opcode=0, uops=lower(spec, ver=ver), rd1_en=True)
        shas[ver] = tmp.sha(ver)
    op = dops.DveOp(name, spec, subdim=False, uops_sha=shas)
    dops.OPS.append(op)
    dops.CUSTOM_DVE_SPECS[name] = spec
    dops._SUB_OPCODE_FOR_NAME[name] = dops._CUSTOM_DVE_ROW_BASE + len(dops.OPS) - 1
    return op


FLOORFIX = _register_floorfix()


def _emit_tile(nc, pools, xt, ot, f=F):
    """Emit the full 64-column computation for one [P, f] x-tile into the
    [P, f*64] out tile. The sin (src=ang) and cos (src=y) chains are emitted
    stage-interleaved so DVE/GP/ACT overlap."""
    f32 = mybir.dt.float32
    i32 = mybir.dt.int32
    A = mybir.AluOpType
    Sin = mybir.ActivationFunctionType.Sin
    Ident = mybir.ActivationFunctionType.Identity
    src_pool, work_pool, hp = pools["src"], pools["work"], pools["hp"]
    gf = G * f

    ov = ot[:].rearrange("p (c e two) -> p two e c", e=D, two=2)

    for g in range(N_GROUPS):
        ang = src_pool.tile([P, gf], f32, tag="src", name="ang%d" % g)
        for i in range(G):
            # NB: single-scalar tensor_scalar on Pool crashes the device;
            # always use the two-op form there.
            if i < 2:
                nc.vector.tensor_scalar_mul(
                    ang[:, i * f:(i + 1) * f], xt[:], float(FREQS[g * G + i]))
            else:
                nc.gpsimd.tensor_scalar(
                    ang[:, i * f:(i + 1) * f], xt[:], 0.0,
                    float(FREQS[g * G + i]), A.add, A.mult)
        y = src_pool.tile([P, gf], f32, tag="src", name="y%d" % g)
        nc.scalar.activation(y[:], ang[:], Ident, bias=hp[:], scale=1.0)
        srcs = (ang, y)
        t_, ki_, kfix_, r_ = {}, {}, {}, {}
        for s in (0, 1):
            t_[s] = work_pool.tile([P, gf], f32, tag="work", name="t%d" % s)
            nc.gpsimd.tensor_scalar(t_[s][:], srcs[s][:], PI, INV2PI,
                                    A.add, A.mult)
        for s in (0, 1):
            ki_[s] = work_pool.tile([P, gf], i32, tag="work", name="ki%d" % s)
            nc.vector.tensor_copy(ki_[s][:], t_[s][:])
        for s in (0, 1):
            kfix_[s] = work_pool.tile([P, gf], f32, tag="work", name="kfix%d" % s)
            nc.vector._custom_dve(FLOORFIX, out=kfix_[s][:], in0=ki_[s][:],
                                  in1=t_[s][:])
        for s in (0, 1):
            r_[s] = work_pool.tile([P, gf], f32, tag="work", name="r%d" % s)
            nc.vector.scalar_tensor_tensor(r_[s][:], kfix_[s][:], NTWOPI,
                                           srcs[s][:], A.mult, A.add)
        for s in (0, 1):
            # sin of ang (s=0) -> odd cols; sin of y (s=1) = cos -> even cols
            nc.scalar.activation(ov[:, 1 - s, g * G:(g + 1) * G, :],
                                 r_[s][:].rearrange("p (e c) -> p e c", e=G),
                                 Sin)


def _emit_kernel(nc, tc, x_v, out_v, n_tiles, repeats=1, loop=None, tiny_ap=None,
                 f=F, bufs_work=16, bufs_src=6):
    f32 = mybir.dt.float32
    with (
        tc.tile_pool(name="io", bufs=2) as io_pool,
        tc.tile_pool(name="src", bufs=bufs_src) as src_pool,
        tc.tile_pool(name="work", bufs=bufs_work) as work_pool,
        tc.tile_pool(name="cst", bufs=1) as cst_pool,
    ):
        hp = cst_pool.tile([P, 1], f32, tag="hp")
        nc.gpsimd.memset(hp[:], HALFPI)
        pools = {"src": src_pool, "work": work_pool, "hp": hp}

        def body():
            xall = io_pool.tile([P, n_tiles * f], f32, tag="xall", bufs=1)
            nc.sync.dma_start(xall[:], x_v)
            for ti in range(n_tiles * repeats):
                tix = ti % n_tiles
                ot = io_pool.tile([P, f * 2 * D], f32, tag="out")
                _emit_tile(nc, pools, xall[:, tix * f:(tix + 1) * f], ot, f=f)
                dma_eng = nc.sync if tix % 2 == 0 else nc.scalar
                dma_eng.dma_start(out_v[tix], ot[:])

        if loop is None:
            body()
        else:
            with tc.For_i(0, loop, 1):
                body()
            xd = io_pool.tile([P, 1], f32, tag="xd")
            nc.vector.memset(xd[:], 0.0)
            nc.sync.dma_start(tiny_ap, xd[:])


def build_nc(repeats: int = 1, n_tiles: int | None = None, n_devices: int = N_CORES,
             f: int = F, bufs_work: int = 16, bufs_src: int = 6):
    f32 = mybir.dt.float32
    if n_tiles is None:
        n_tiles = N_PER_CORE // (P * f)
    n_elems = n_tiles * P * f
    nc = bacc.Bacc("TRN2", target_bir_lowering=False, debug=False,
                   num_devices=n_devices)
    x = nc.dram_tensor("x", [n_elems], f32, kind="ExternalInput")
    out = nc.dram_tensor("out", [n_elems, 2 * D], f32, kind="ExternalOutput")
    x_v = x.ap().rearrange("(p w) -> p w", p=P)
    out_v = out.ap().rearrange("(p t c) q -> t p (c q)", p=P, t=n_tiles)
    with tile.TileContext(nc) as tc:
        _emit_kernel(nc, tc, x_v, out_v, n_tiles, repeats=repeats,
                     f=f, bufs_work=bufs_work, bufs_src=bufs_src)
    nc.compile()
    return nc


def build_timing_nc(loop_iters: int, n_tiles: int | None = None,
                    f: int = F, bufs_work: int = 16, bufs_src: int = 6):
    """Timing variant: out goes to internal DRAM; whole pass loops on-device;
    only a tiny dummy tensor is fetched back."""
    f32 = mybir.dt.float32
    if n_tiles is None:
        n_tiles = N_PER_CORE // (P * f)
    n_elems = n_tiles * P * f
    nc = bacc.Bacc("TRN2", target_bir_lowering=False, debug=False,
                   num_devices=N_CORES)
    x = nc.dram_tensor("x", [n_elems], f32, kind="ExternalInput")
    out = nc.dram_tensor("scratch_out", [n_elems, 2 * D], f32)
    tiny = nc.dram_tensor("tiny_out", [P, 1], f32, kind="ExternalOutput")
    x_v = x.ap().rearrange("(p w) -> p w", p=P)
    out_v = out.ap().rearrange("(p t c) q -> t p (c q)", p=P, t=n_tiles)
    with tile.TileContext(nc) as tc:
        _emit_kernel(nc, tc, x_v, out_v, n_tiles, loop=loop_iters, tiny_ap=tiny.ap(),
                     f=f, bufs_work=bufs_work, bufs_src=bufs_src)
    nc.compile()
    return nc


def _get_nc(repeats: int = 1):
    if repeats not in _NC_CACHE:
        _NC_CACHE[repeats] = build_nc(repeats)
    return _NC_CACHE[repeats]


def kernel(x, d):
    assert int(d) == D
    x = np.ascontiguousarray(np.asarray(x, dtype=np.float32).reshape(N_TOTAL))
    xs = x.reshape(N_CORES, N_PER_CORE)
    nc = _get_nc()
    res = run_bass_kernel_spmd(
        nc, [{"x": xs[i]} for i in range(N_CORES)], core_ids=list(range(N_CORES)))
    out = np.empty((N_TOTAL, 2 * D), dtype=np.float32)
    for i in range(N_CORES):
        out[i * N_PER_CORE:(i + 1) * N_PER_CORE] = res.results[i]["out"]
    return out
